# revision 1
# baseline (speedup 1.0000x reference)
"""Trainium2 Bass kernel for a dense transformer block.

Reference computation (per batch element):
    y  = Attention(LN1(x)) ; x = x + y
    x  = x + MLP(LN2(x))
with B=8, N=1024, C=768, H=12 heads, head_dim=64, HIDDEN=3072, fp32 I/O.

Sharding: data-parallel over B across the 8 NeuronCores — each core runs the
full block on one (1024, 768) batch element with replicated weights. No
collectives.

Per-core design notes:
  * Matmul operands are bf16 (weights pre-cast on host); PSUM accumulation and
    the residual stream / layernorm statistics stay fp32.
  * Activations are kept token-major ([tokens->partitions, feats->free]) for
    layernorm + residuals, and feature-major (x_lnT) as the matmul lhsT /
    rhs. The feature-major copies are produced with DMA transposes (bf16).
  * Attention computes S^T = K^T.T @ Q^T per (head, key-tile) so softmax
    probabilities land with k_tokens on partitions, which is exactly the
    layout the attention@V matmul needs as rhs. Softmax denominators come for
    free from a ones-column appended to the (token-major) V tile: the AV
    matmul's 65th output row is the per-query sum of exp-scores.
  * The 1/8 attention scale is folded into the Exp activation's scale input;
    max-subtraction is skipped (scores for this problem are < ~2 in
    magnitude, far from exp overflow).
"""

import os

import numpy as np
import ml_dtypes

import concourse.bass as bass
import concourse.bacc as bacc
import concourse.mybir as mybir
import concourse.tile as tile
from concourse import bass_utils

# Model dims (hardcoded per the problem spec).
B = 8
N = 1024  # tokens
C = 768  # model dim
H = 12  # heads
HD = 64  # head dim
HID = 3072  # mlp hidden
EPS = 1e-5
P = 128  # SBUF partitions

NT = N // P  # 8 token tiles
KC = C // P  # 6 contraction tiles over C
KH = HID // P  # 24 contraction tiles over HIDDEN

F32 = mybir.dt.float32
BF16 = mybir.dt.bfloat16
AF = mybir.ActivationFunctionType
ALU = mybir.AluOpType

_cache = {}

# CoreSim doesn't implement the Gelu activation table; when True the builder
# emits a tanh-approximation decomposition instead (dev/validation only).
SIM_GELU = False


def _build(flags):
    """Trace the per-core Bass program. `flags` gates optional bias/gain work."""
    (use_bqkv, use_g1, use_beta1, use_g2, use_beta2, use_bfc1, use_bproj,
     use_bfc2) = flags

    nc = bacc.Bacc("TRN2", target_bir_lowering=False, debug=False)

    x_d = nc.dram_tensor("x", [N, C], F32, kind="ExternalInput")
    wqkv_d = nc.dram_tensor("wqkv", [C, 3 * C], BF16, kind="ExternalInput")
    wproj_d = nc.dram_tensor("wproj", [C, C], BF16, kind="ExternalInput")
    wfc1_d = nc.dram_tensor("wfc1", [C, HID], BF16, kind="ExternalInput")
    wfc2_d = nc.dram_tensor("wfc2", [HID, C], BF16, kind="ExternalInput")
    out_d = nc.dram_tensor("out", [N, C], F32, kind="ExternalOutput")

    opt_d = {}
    for name, use, shape in (
        ("bqkv", use_bqkv, [3 * C]),
        ("g1", use_g1, [C]),
        ("beta1", use_beta1, [C]),
        ("g2", use_g2, [C]),
        ("beta2", use_beta2, [C]),
        ("bfc1", use_bfc1, [HID]),
        ("bproj", use_bproj, [C]),
        ("bfc2", use_bfc2, [C]),
    ):
        if use:
            opt_d[name] = nc.dram_tensor(name, shape, F32, kind="ExternalInput")

    def bcast_from_dram(pool, ap_1d, n):
        """[n] DRAM vector -> [P, n] SBUF tile replicated on every partition."""
        t = pool.tile([P, n], F32, name=f"bc_{ap_1d.tensor.name}")
        src = bass.AP(tensor=ap_1d.tensor, offset=ap_1d.offset,
                      ap=[[0, P]] + list(ap_1d.ap))
        nc.sync.dma_start(out=t, in_=src)
        return t

    with tile.TileContext(nc) as tc:
        persist = tc.alloc_tile_pool(name="persist", bufs=1, side="left")
        psum = tc.alloc_tile_pool(name="psum", bufs=1, space="PSUM")
        dram = tc.alloc_tile_pool(name="dram", bufs=2, space="DRAM")

        # Residual stream, token-major; updated in place through the block.
        x_sb = persist.tile([P, NT, C], F32)
        for t in range(NT):
            nc.sync.dma_start(out=x_sb[:, t, :], in_=x_d.ap()[t * P:(t + 1) * P, :])
        eps_t = persist.tile([P, 1], F32)
        nc.vector.memset(eps_t, EPS)

        # Identity (bf16, embedded in the NEFF) for PE-based transposes.
        ident_d = nc.inline_tensor(np.eye(P, dtype=ml_dtypes.bfloat16), "ident")
        ident = persist.tile([P, P], BF16)
        nc.sync.dma_start(out=ident, in_=ident_d.ap())

        g_beta = {}
        for name, n in (("g1", C), ("beta1", C), ("g2", C), ("beta2", C),
                        ("bproj", C), ("bfc2", C)):
            if name in opt_d:
                g_beta[name] = bcast_from_dram(persist, opt_d[name].ap(), n)
        bqkv_sb = None
        if "bqkv" in opt_d:
            bqkv_sb = persist.tile([P, 3 * C // P], F32)
            nc.sync.dma_start(out=bqkv_sb,
                              in_=opt_d["bqkv"].ap().rearrange("(m p) -> p m", p=P))
        bfc1_sb = None
        if "bfc1" in opt_d:
            bfc1_sb = persist.tile([P, KH], F32)
            nc.sync.dma_start(out=bfc1_sb,
                              in_=opt_d["bfc1"].ap().rearrange("(m p) -> p m", p=P))

        # ---------------------------------------------------------------
        # Phase 1: LN1 (token-major) -> x_lnT (feature-major bf16), weights
        # ---------------------------------------------------------------
        p1 = tc.alloc_tile_pool(name="p1", bufs=1, side="left")
        ln1 = tc.alloc_tile_pool(name="ln1", bufs=3, side="left")

        wqkv_sb = p1.tile([P, KC, 3 * C], BF16)
        for ko in range(KC):
            nc.sync.dma_start(out=wqkv_sb[:, ko, :],
                              in_=wqkv_d.ap()[ko * P:(ko + 1) * P, :])

        xlnT = p1.tile([P, KC, N], BF16)

        def layernorm_tile(pool, x_ap, g_sb, beta_sb, name):
            """x_ap: [P, C] fp32 token-major -> returns [P, C] bf16 tile."""
            stats = pool.tile([P, 3, 6], F32, tag=f"{name}_st", bufs=3)
            xr = x_ap.rearrange("p (s f) -> p s f", f=256)
            for s in range(3):
                nc.vector.bn_stats(out=stats[:, s, :], in_=xr[:, s, :])
            mv = pool.tile([P, 2], F32, tag=f"{name}_mv", bufs=3)
            nc.vector.bn_aggr(out=mv, in_=stats)
            rstd = pool.tile([P, 1], F32, tag=f"{name}_rs", bufs=3)
            nc.scalar.activation(out=rstd, in_=mv[:, 1:2], func=AF.Sqrt,
                                 bias=eps_t, scale=1.0)
            nc.vector.reciprocal(out=rstd, in_=rstd)
            xln = pool.tile([P, C], BF16, tag=f"{name}_xln", bufs=3)
            nc.vector.tensor_scalar(out=xln, in0=x_ap, scalar1=mv[:, 0:1],
                                    scalar2=rstd, op0=ALU.subtract, op1=ALU.mult)
            if g_sb is not None:
                nc.vector.tensor_mul(out=xln, in0=xln, in1=g_sb)
            if beta_sb is not None:
                nc.vector.tensor_add(out=xln, in0=xln, in1=beta_sb)
            return xln

        def transpose_to(xln, dstT, t):
            """[P, C] token-major tile -> dstT[:, :, t*P:(t+1)*P] feature-major."""
            for c in range(KC):
                tps = psum.tile([P, P], BF16, tag="av", bufs=4, name="tps")
                nc.tensor.transpose(tps, xln[:, c * P:(c + 1) * P], ident)
                nc.scalar.copy(out=dstT[:, c, t * P:(t + 1) * P], in_=tps)

        for t in range(NT):
            xln = layernorm_tile(ln1, x_sb[:, t, :], g_beta.get("g1"),
                                 g_beta.get("beta1"), "ln1")
            transpose_to(xln, xlnT, t)

        # ---------------------------------------------------------------
        # Phase 2: qkv projections.
        #   q^T,k^T feature-major: [2C, N] as 12 tiles of [128, N]
        #   V token-major with ones column: V_aug [P, NT, H, HD+1]
        # ---------------------------------------------------------------
        p2 = tc.alloc_tile_pool(name="p2", bufs=1, side="right")
        qkT = p2.tile([P, 2 * KC, N], BF16)
        v_aug = p2.tile([P, NT, H, HD + 1], BF16)
        nc.vector.memset(v_aug[:, :, :, HD:HD + 1], 1.0)

        # q^T / k^T: out^T[m-block, tokens] = wqkv[:, m-block].T @ x_ln^T
        for m in range(2 * KC):
            for n0 in range(0, N, 512):
                ps = psum.tile([P, 512], F32, tag="mm", bufs=4, name="ps_mm")
                for ko in range(KC):
                    nc.tensor.matmul(ps, wqkv_sb[:, ko, m * P:(m + 1) * P],
                                     xlnT[:, ko, n0:n0 + 512],
                                     start=(ko == 0), stop=(ko == KC - 1))
                if bqkv_sb is not None:
                    nc.any.tensor_scalar_add(qkT[:, m, n0:n0 + 512], ps,
                                             bqkv_sb[:, m:m + 1])
                else:
                    nc.vector.tensor_copy(out=qkT[:, m, n0:n0 + 512], in_=ps)

        # V token-major: V[tok-tile, vfeats] = x_ln @ wqkv[:, 2C:3C]
        for t in range(NT):
            for j, (n0, nn) in enumerate(((0, 512), (512, 256))):
                ps = psum.tile([P, 512], F32, tag="mm", bufs=4, name="ps_mm")[:, :nn]
                for ko in range(KC):
                    nc.tensor.matmul(ps, xlnT[:, ko, t * P:(t + 1) * P],
                                     wqkv_sb[:, ko, 2 * C + n0:2 * C + n0 + nn],
                                     start=(ko == 0), stop=(ko == KC - 1))
                # scatter heads into the 65-strided V_aug layout
                nh = nn // HD
                dst = v_aug[:, t, j * 8:j * 8 + nh, 0:HD]
                if bqkv_sb is not None:
                    # per-feature bias along free dim: use broadcast tile
                    bq = g_beta.get("bqkv_v")
                    if bq is None:
                        bq = bcast_from_dram(persist, opt_d["bqkv"].ap()[2 * C:3 * C], C)
                        g_beta["bqkv_v"] = bq
                    nc.any.tensor_add(out=dst,
                                      in0=ps.rearrange("p (h d) -> p h d", d=HD),
                                      in1=bq[:, n0:n0 + nn].rearrange(
                                          "p (h d) -> p h d", d=HD))
                else:
                    nc.vector.tensor_copy(
                        out=dst, in_=ps.rearrange("p (h d) -> p h d", d=HD))

        ln1.release()
        p1.release()

        # ---------------------------------------------------------------
        # Phase 3: attention, head by head.
        # ---------------------------------------------------------------
        p3 = tc.alloc_tile_pool(name="p3", bufs=1, side="left")
        att = tc.alloc_tile_pool(name="att", bufs=1, side="left")
        attnT = p3.tile([P, KC, N], BF16)
        wproj_sb = p3.tile([P, KC, C], BF16)
        for ko in range(KC):
            nc.sync.dma_start(out=wproj_sb[:, ko, :],
                              in_=wproj_d.ap()[ko * P:(ko + 1) * P, :])

        # Heads are processed in pairs with burst scheduling: all 16 S^T
        # matmuls of a pair are emitted back-to-back (one long PE burst, with
        # the Exp evictions trailing on the Scalar engine), then all 32 AV
        # matmuls. Long uninterrupted bursts keep the PE's HAM clock-gate at
        # full rate; fine-grained S->exp->AV interleaving leaves it throttled.
        for i in range(H // 2):
            es_store = {}
            for h in (2 * i, 2 * i + 1):
                pb = (h % 2) * HD
                qT = qkT[pb:pb + HD, h // 2, :]
                kT = qkT[pb:pb + HD, KC + h // 2, :]
                for kt in range(NT):
                    es = att.tile([P, N], BF16, tag="es", bufs=18,
                                  name=f"es_{h}_{kt}")
                    for j in range(2):
                        sps = psum.tile([P, 512], F32, tag="mm", bufs=4,
                                        name=f"s{j}_{h}_{kt}")
                        nc.tensor.matmul(sps, kT[:, kt * P:(kt + 1) * P],
                                         qT[:, j * 512:(j + 1) * 512],
                                         start=True, stop=True)
                        nc.scalar.activation(out=es[:, j * 512:(j + 1) * 512],
                                             in_=sps, func=AF.Exp, scale=0.125)
                    es_store[h, kt] = es

            avs = {}
            for h in (2 * i, 2 * i + 1):
                avs[h] = [psum.tile([HD + 1, 512], F32, tag="av", bufs=4,
                                    name=f"av{j}_{h}") for j in range(2)]
                for kt in range(NT):
                    for j in range(2):
                        nc.tensor.matmul(avs[h][j], v_aug[:, kt, h, :],
                                         es_store[h, kt][:, j * 512:(j + 1) * 512],
                                         start=(kt == 0), stop=(kt == NT - 1))

            for h in (2 * i, 2 * i + 1):
                pb = (h % 2) * HD
                av = avs[h]
                # Evict accumulators to SBUF right away (fast full-width DVE
                # copies) so the PSUM banks free up for the next pair.
                av_sb = att.tile([HD + 1, N], F32, tag="avsb", bufs=3,
                                 name=f"avsb_{h}")
                for j in range(2):
                    nc.scalar.copy(out=av_sb[:, j * 512:(j + 1) * 512],
                                   in_=av[j])
                # Softmax denominators: row HD holds sum_k exp(S). Broadcast
                # to partitions 0..HD-1 via a DRAM bounce (step-0 partition
                # reads are only legal from DRAM), then reciprocal at full
                # partition width and normalize.
                rdram = dram.tile([1, N], F32, tag="rdram", bufs=2)
                nc.sync.dma_start(out=rdram, in_=av_sb[HD:HD + 1, :])
                rbc = att.tile([HD, N], F32, tag="rbc", bufs=2, name=f"rbc{h}")
                rd = rdram[0, :]
                rbc_src = bass.AP(tensor=rd.tensor, offset=rd.offset,
                                  ap=[[0, HD]] + list(rd.ap))
                nc.sync.dma_start(out=rbc, in_=rbc_src)
                nc.vector.reciprocal(out=rbc, in_=rbc)
                bounce = att.tile([HD, N], BF16, tag="bounce", bufs=2,
                                  name=f"bounce{h}")
                nc.vector.tensor_mul(out=bounce, in0=av_sb[0:HD, :], in1=rbc)
                nc.sync.dma_start(out=attnT[pb:pb + HD, h // 2, :], in_=bounce)

        p2.release()

        # ---------------------------------------------------------------
        # Phase 4: proj + residual, LN2 -> x2_lnT
        # ---------------------------------------------------------------
        p4 = tc.alloc_tile_pool(name="p4", bufs=1, side="right")
        ln2 = tc.alloc_tile_pool(name="ln2", bufs=3, side="right")
        x2lnT = p4.tile([P, KC, N], BF16)
        wfc1_sb = p4.tile([P, KC, HID], BF16)
        for ko in range(KC):
            nc.sync.dma_start(out=wfc1_sb[:, ko, :],
                              in_=wfc1_d.ap()[ko * P:(ko + 1) * P, :])

        for t in range(NT):
            for n0, nn in ((0, 512), (512, 256)):
                ps = psum.tile([P, 512], F32, tag="mm", bufs=4, name="ps_mm")[:, :nn]
                for ko in range(KC):
                    nc.tensor.matmul(ps, attnT[:, ko, t * P:(t + 1) * P],
                                     wproj_sb[:, ko, n0:n0 + nn],
                                     start=(ko == 0), stop=(ko == KC - 1))
                xs = x_sb[:, t, n0:n0 + nn]
                nc.vector.tensor_add(out=xs, in0=xs, in1=ps)
                if "bproj" in g_beta:
                    nc.vector.tensor_add(out=xs, in0=xs,
                                         in1=g_beta["bproj"][:, n0:n0 + nn])
            xln = layernorm_tile(ln2, x_sb[:, t, :], g_beta.get("g2"),
                                 g_beta.get("beta2"), "ln2")
            transpose_to(xln, x2lnT, t)

        att.release()
        p3.release()

        # ---------------------------------------------------------------
        # Phase 5: fc1 + gelu -> h^T (feature-major bf16)
        # ---------------------------------------------------------------
        p5 = tc.alloc_tile_pool(name="p5", bufs=1, side="left")
        hT = p5.tile([P, KH, N], BF16)
        wfc2_sb = p5.tile([P, KH, C], BF16)
        for ko in range(KH):
            nc.sync.dma_start(out=wfc2_sb[:, ko, :],
                              in_=wfc2_d.ap()[ko * P:(ko + 1) * P, :])

        for m in range(KH):
            for n0 in range(0, N, 512):
                ps = psum.tile([P, 512], F32, tag="mm", bufs=4, name="ps_mm")
                for ko in range(KC):
                    nc.tensor.matmul(ps, wfc1_sb[:, ko, m * P:(m + 1) * P],
                                     x2lnT[:, ko, n0:n0 + 512],
                                     start=(ko == 0), stop=(ko == KC - 1))
                bias = bfc1_sb[:, m:m + 1] if bfc1_sb is not None else 0.0
                if not SIM_GELU:
                    nc.scalar.activation(out=hT[:, m, n0:n0 + 512], in_=ps,
                                         func=AF.Gelu, bias=bias, scale=1.0)
                else:
                    # gelu(x) ~= 0.5 x (1 + tanh(sqrt(2/pi)(x + 0.044715 x^3)))
                    a = ln2.tile([P, 512], F32, tag="g_a", bufs=2)
                    nc.scalar.activation(out=a, in_=ps, func=AF.Copy,
                                         bias=0.0, scale=1.0)
                    if bfc1_sb is not None:
                        nc.vector.tensor_scalar_add(a, a, bfc1_sb[:, m:m + 1])
                    u = ln2.tile([P, 512], F32, tag="g_u", bufs=2)
                    nc.vector.tensor_mul(out=u, in0=a, in1=a)
                    nc.vector.tensor_mul(out=u, in0=u, in1=a)
                    nc.vector.tensor_scalar_mul(u, u, 0.044715)
                    nc.vector.tensor_add(out=u, in0=u, in1=a)
                    nc.scalar.activation(out=u, in_=u, func=AF.Tanh,
                                         bias=0.0, scale=0.7978845608028654)
                    nc.vector.tensor_scalar_add(u, u, 1.0)
                    nc.vector.tensor_scalar_mul(a, a, 0.5)
                    nc.vector.tensor_mul(out=hT[:, m, n0:n0 + 512],
                                         in0=a, in1=u)

        ln2.release()
        p4.release()

        # ---------------------------------------------------------------
        # Phase 6: fc2 + residual -> out
        # ---------------------------------------------------------------
        for t in range(NT):
            for n0, nn in ((0, 512), (512, 256)):
                ps = psum.tile([P, 512], F32, tag="mm", bufs=4, name="ps_mm")[:, :nn]
                for ko in range(KH):
                    nc.tensor.matmul(ps, hT[:, ko, t * P:(t + 1) * P],
                                     wfc2_sb[:, ko, n0:n0 + nn],
                                     start=(ko == 0), stop=(ko == KH - 1))
                xs = x_sb[:, t, n0:n0 + nn]
                nc.vector.tensor_add(out=xs, in0=xs, in1=ps)
                if "bfc2" in g_beta:
                    nc.vector.tensor_add(out=xs, in0=xs,
                                         in1=g_beta["bfc2"][:, n0:n0 + nn])
            nc.sync.dma_start(out=out_d.ap()[t * P:(t + 1) * P, :],
                              in_=x_sb[:, t, :])

        p5.release()
        persist.release()
        dram.release()
        psum.release()

    nc.compile()
    return nc


def _prep(inputs):
    """Host-side prep: shard x over B, cast weights to bf16, compute gates."""
    f = {k: np.asarray(v) for k, v in inputs.items()}
    bf = ml_dtypes.bfloat16

    flags = (
        bool(np.any(f["b_qkv"])),
        not np.all(f["g1"] == 1.0),
        bool(np.any(f["beta1"])),
        not np.all(f["g2"] == 1.0),
        bool(np.any(f["beta2"])),
        bool(np.any(f["b_fc1"])),
        bool(np.any(f["b_proj"])),
        bool(np.any(f["b_fc2"])),
    )
    (use_bqkv, use_g1, use_beta1, use_g2, use_beta2, use_bfc1, use_bproj,
     use_bfc2) = flags

    common = {
        "wqkv": np.ascontiguousarray(f["w_qkv"].astype(bf)),
        "wproj": np.ascontiguousarray(f["w_proj"].astype(bf)),
        "wfc1": np.ascontiguousarray(f["w_fc1"].astype(bf)),
        "wfc2": np.ascontiguousarray(f["w_fc2"].astype(bf)),
    }
    for name, key, use in (
        ("bqkv", "b_qkv", use_bqkv), ("g1", "g1", use_g1),
        ("beta1", "beta1", use_beta1), ("g2", "g2", use_g2),
        ("beta2", "beta2", use_beta2), ("bfc1", "b_fc1", use_bfc1),
        ("bproj", "b_proj", use_bproj), ("bfc2", "b_fc2", use_bfc2),
    ):
        if use:
            common[name] = np.ascontiguousarray(f[key].astype(np.float32))

    x = f["x"].astype(np.float32)
    in_maps = [dict(common, x=np.ascontiguousarray(x[i])) for i in range(B)]
    return flags, in_maps


LAST_RESULT = None


def kernel(**inputs):
    global LAST_RESULT
    flags, in_maps = _prep(inputs)
    if flags not in _cache:
        _cache[flags] = _build(flags)
    nc = _cache[flags]
    res = bass_utils.run_bass_kernel_spmd(nc, in_maps, core_ids=list(range(B)))
    LAST_RESULT = res
    out = np.stack([r["out"] for r in res.results], axis=0)
    return out.astype(np.float32)



# revision 6
# speedup vs baseline: 1.2624x; 1.2624x over previous
"""Trainium2 Bass kernel for a dense transformer block.

Reference computation (per batch element):
    y  = Attention(LN1(x)) ; x = x + y
    x  = x + MLP(LN2(x))
with B=8, N=1024, C=768, H=12 heads, head_dim=64, HIDDEN=3072, fp32 I/O.

Sharding: data-parallel over B across the 8 NeuronCores — each core runs the
full block on one (1024, 768) batch element with replicated weights. No
collectives.

Per-core design notes:
  * Most matmuls run in fp8 (TRN FP8_EXP4 = e4m3, max +-240) with
    MatmulPerfMode.DoubleRow: operands carry TWO contraction chunks along a
    size-2 free dim ([K, 2, *]), so one instruction contracts 256 elements
    at bf16 instruction cost -> 2x PE throughput. Weights are pre-scaled by
    512 on the host (centers the 0.02-std weights in e4m3 range); the 1/512
    descale folds into each eviction (activation scale / tensor_scalar
    mult). fp8 matmuls: qkv, V, attention AV, proj, fc1.
  * S = q.k^T stays bf16 (its contraction dim is 64 - no pair layout without
    a partition shuffle) and fc2 stays bf16 for accuracy (fp8 fc1+fc2
    together would eat most of the 2e-2 error budget; fc1-only sims at
    1.33e-2 vs the 2e-2 gate).
  * Softmax: exp evictions are [128, 1024] activations spanning two PSUM
    banks; exp emits 64*exp(S/8) directly into fp8 (1/8 folded into the
    activation scale, the x64 into its bias = ln 64) so probabilities sit
    mid-range in e4m3. The AV matmul's 65th output row (from a ones-column
    in V) gives denominators; reciprocals are BATCHED across 6 heads into
    one [6, 1024] DVE reciprocal (DVE reciprocal cost depends only on free
    size - per-head [64, N] reciprocals waste 6.6us each).
  * qkT production is interleaved with per-head attention so the scalar
    engine's exp stream (the attention-phase bottleneck, ~100us) starts as
    soon as head 0's q/k slices exist instead of after all 12 projections.
  * Activations are token-major for layernorm/residuals; feature-major
    matmul operands come from PE transposes (bf16) whose evictions cast to
    fp8.
"""

import numpy as np
import ml_dtypes

import concourse.bass as bass
import concourse.bacc as bacc
import concourse.mybir as mybir
import concourse.tile as tile
from concourse import bass_utils

# Model dims (hardcoded per the problem spec).
B = 8
N = 1024  # tokens
C = 768  # model dim
H = 12  # heads
HD = 64  # head dim
HID = 3072  # mlp hidden
EPS = 1e-5
P = 128  # SBUF partitions

NT = N // P  # 8 token tiles
KC = C // P  # 6 contraction tiles over C
KP = KC // 2  # 3 DoubleRow pairs over C
KH = HID // P  # 24 contraction tiles over HIDDEN

WS = 512.0  # host-side weight scale for fp8 quantization
# Probability scale folded into the exp bias (probs = PS * exp(S/8) in fp8).
# Must keep PS * exp(max |S|/8) well under 240: TRN fp8 SATURATES TO INF, so
# an overflowing prob poisons the whole row. PS=8 trips only beyond ~11
# sigma of the score distribution while keeping typical probs ~[2, 30],
# mid-range in e4m3.
PS = 8.0

F32 = mybir.dt.float32
BF16 = mybir.dt.bfloat16
FP8 = mybir.dt.float8e4
AF = mybir.ActivationFunctionType
ALU = mybir.AluOpType
DR = mybir.MatmulPerfMode.DoubleRow

_cache = {}


def _build(flags):
    """Trace the per-core Bass program. `flags` gates optional bias/gain work."""
    (use_bqkv, use_g1, use_beta1, use_g2, use_beta2, use_bfc1, use_bproj,
     use_bfc2) = flags

    nc = bacc.Bacc("TRN2", target_bir_lowering=False, debug=False)

    x_d = nc.dram_tensor("x", [N, C], F32, kind="ExternalInput")
    wqkv_d = nc.dram_tensor("wqkv", [C, 3 * C], FP8, kind="ExternalInput")
    wproj_d = nc.dram_tensor("wproj", [C, C], FP8, kind="ExternalInput")
    wfc1_d = nc.dram_tensor("wfc1", [C, HID], FP8, kind="ExternalInput")
    wfc2_d = nc.dram_tensor("wfc2", [HID, C], BF16, kind="ExternalInput")
    out_d = nc.dram_tensor("out", [N, C], F32, kind="ExternalOutput")

    opt_d = {}
    for name, use, shape in (
        ("bqkv", use_bqkv, [3 * C]),
        ("g1", use_g1, [C]),
        ("beta1", use_beta1, [C]),
        ("g2", use_g2, [C]),
        ("beta2", use_beta2, [C]),
        ("bfc1", use_bfc1, [HID]),
        ("bproj", use_bproj, [C]),
        ("bfc2", use_bfc2, [C]),
    ):
        if use:
            opt_d[name] = nc.dram_tensor(name, shape, F32, kind="ExternalInput")

    def bcast_from_dram(pool, ap_1d, n):
        """[n] DRAM vector -> [P, n] SBUF tile replicated on every partition."""
        t = pool.tile([P, n], F32, name=f"bc_{ap_1d.tensor.name}")
        src = bass.AP(tensor=ap_1d.tensor, offset=ap_1d.offset,
                      ap=[[0, P]] + list(ap_1d.ap))
        nc.sync.dma_start(out=t, in_=src)
        return t

    with tile.TileContext(nc) as tc:
        persist = tc.alloc_tile_pool(name="persist", bufs=1, side="left")
        psum = tc.alloc_tile_pool(name="psum", bufs=1, space="PSUM")
        dram = tc.alloc_tile_pool(name="dram", bufs=2, space="DRAM")

        # Residual stream, token-major; updated in place through the block.
        x_sb = persist.tile([P, NT, C], F32)
        for t in range(NT):
            nc.sync.dma_start(out=x_sb[:, t, :], in_=x_d.ap()[t * P:(t + 1) * P, :])
        eps_t = persist.tile([P, 1], F32)
        nc.vector.memset(eps_t, EPS)
        ln64_t = persist.tile([P, 1], F32)
        nc.vector.memset(ln64_t, float(np.log(PS)))

        # Identity (bf16, embedded in the NEFF) for PE-based transposes.
        ident_d = nc.inline_tensor(np.eye(P, dtype=ml_dtypes.bfloat16), "ident")
        ident = persist.tile([P, P], BF16)
        nc.sync.dma_start(out=ident, in_=ident_d.ap())

        g_beta = {}
        for name, n in (("g1", C), ("beta1", C), ("g2", C), ("beta2", C),
                        ("bproj", C), ("bfc2", C)):
            if name in opt_d:
                g_beta[name] = bcast_from_dram(persist, opt_d[name].ap(), n)
        bqkv_sb = None
        if "bqkv" in opt_d:
            bqkv_sb = persist.tile([P, 3 * C // P], F32)
            nc.sync.dma_start(out=bqkv_sb,
                              in_=opt_d["bqkv"].ap().rearrange("(m p) -> p m", p=P))
        bfc1_sb = None
        if "bfc1" in opt_d:
            bfc1_sb = persist.tile([P, KH], F32)
            nc.sync.dma_start(out=bfc1_sb,
                              in_=opt_d["bfc1"].ap().rearrange("(m p) -> p m", p=P))

        # ---------------------------------------------------------------
        # Phase 1: LN1 (token-major) -> x_lnT (feature-major fp8), weights
        # ---------------------------------------------------------------
        p1 = tc.alloc_tile_pool(name="p1", bufs=1, side="left")
        ln1 = tc.alloc_tile_pool(name="ln1", bufs=3, side="left")

        wqkv_sb = p1.tile([P, KC, 3 * C], FP8)
        for ko in range(KC):
            nc.sync.dma_start(out=wqkv_sb[:, ko, :],
                              in_=wqkv_d.ap()[ko * P:(ko + 1) * P, :])

        xlnT = p1.tile([P, KC, N], FP8)

        def layernorm_tile(pool, x_ap, g_sb, beta_sb, name):
            """x_ap: [P, C] fp32 token-major -> returns [P, C] bf16 tile."""
            stats = pool.tile([P, 3, 6], F32, tag=f"{name}_st", bufs=3)
            xr = x_ap.rearrange("p (s f) -> p s f", f=256)
            for s in range(3):
                nc.vector.bn_stats(out=stats[:, s, :], in_=xr[:, s, :])
            mv = pool.tile([P, 2], F32, tag=f"{name}_mv", bufs=3)
            nc.vector.bn_aggr(out=mv, in_=stats)
            rstd = pool.tile([P, 1], F32, tag=f"{name}_rs", bufs=3)
            nc.scalar.activation(out=rstd, in_=mv[:, 1:2], func=AF.Sqrt,
                                 bias=eps_t, scale=1.0)
            nc.vector.reciprocal(out=rstd, in_=rstd)
            xln = pool.tile([P, C], BF16, tag=f"{name}_xln", bufs=3)
            nc.vector.tensor_scalar(out=xln, in0=x_ap, scalar1=mv[:, 0:1],
                                    scalar2=rstd, op0=ALU.subtract, op1=ALU.mult)
            if g_sb is not None:
                nc.vector.tensor_mul(out=xln, in0=xln, in1=g_sb)
            if beta_sb is not None:
                nc.vector.tensor_add(out=xln, in0=xln, in1=beta_sb)
            return xln

        def transpose_to(xln, dstT, t):
            """[P, C] token-major bf16 tile -> dstT[:, :, t*P:(t+1)*P] fp8."""
            for c in range(KC):
                tps = psum.tile([P, P], BF16, tag="avp", bufs=2, name="tps")
                nc.tensor.transpose(tps, xln[:, c * P:(c + 1) * P], ident)
                nc.scalar.copy(out=dstT[:, c, t * P:(t + 1) * P], in_=tps)

        for t in range(NT):
            xln = layernorm_tile(ln1, x_sb[:, t, :], g_beta.get("g1"),
                                 g_beta.get("beta1"), "ln1")
            transpose_to(xln, xlnT, t)

        # ---------------------------------------------------------------
        # Phase 2: V projection first (fp8 DoubleRow), then qkT interleaved
        # with attention heads.
        #   V token-major fp8, kt-paired with a ones column:
        #     v_aug [P, NT/2, 2, H*(HD+1)]
        # ---------------------------------------------------------------
        p2 = tc.alloc_tile_pool(name="p2", bufs=1, side="right")
        qkT = p2.tile([P, 2 * KC, N], BF16)
        # Head blocks padded to 80 bytes: dual-fp8 LDWEIGHTS requires the
        # pair-dim byte stride (2*H*HB) and offsets to be 16-aligned.
        HB = 80  # 64 v-cols + 1 ones column + 15 pad
        v_aug = p2.tile([P, NT // 2, 2, H * HB], FP8)
        ones_view = v_aug.rearrange("p a b (h d) -> p (a b h) d", d=HB)
        nc.vector.memset(ones_view[:, :, HD:HD + 1], 1.0)

        bq_v = None
        if use_bqkv:
            bq_v = bcast_from_dram(persist, opt_d["bqkv"].ap()[2 * C:3 * C], C)

        for t in range(NT):
            ps = psum.tile([P, 1024], F32, tag="mm", bufs=2, name="ps_v")
            for n0, nn in ((0, 512), (512, 256)):
                for c in range(KP):
                    nc.tensor.matmul(ps[:, n0:n0 + nn],
                                     xlnT[:, 2 * c:2 * c + 2, t * P:(t + 1) * P],
                                     wqkv_sb[:, 2 * c:2 * c + 2,
                                             2 * C + n0:2 * C + n0 + nn],
                                     start=(c == 0), stop=(c == KP - 1),
                                     perf_mode=DR)
            # scatter heads into the pair-major V_aug layout (fp8, 1/WS)
            dst = v_aug[:, t // 2, t % 2, :].rearrange(
                "p (h d) -> p h d", d=HB)[:, :, 0:HD]
            if bq_v is not None:
                tmp = ln1.tile([P, C], F32, tag="vtmp", bufs=2)
                nc.vector.tensor_scalar_mul(tmp, ps[:, 0:C], 1.0 / WS)
                nc.vector.tensor_add(out=tmp, in0=tmp, in1=bq_v)
                nc.vector.tensor_copy(
                    out=dst, in_=tmp.rearrange("p (h d) -> p h d", d=HD))
            else:
                nc.vector.tensor_scalar_mul(
                    dst, ps[:, 0:C].rearrange("p (h d) -> p h d", d=HD),
                    1.0 / WS)

        def project_qk(m):
            """qkT[:, m, :] = (wqkv[:, m-block].T @ x_ln^T) / WS (+bias)."""
            ps = psum.tile([P, 1024], F32, tag="mm", bufs=2, name="ps_qk")
            for n0 in (0, 512):
                for c in range(KP):
                    nc.tensor.matmul(ps[:, n0:n0 + 512],
                                     wqkv_sb[:, 2 * c:2 * c + 2,
                                             m * P:(m + 1) * P],
                                     xlnT[:, 2 * c:2 * c + 2, n0:n0 + 512],
                                     start=(c == 0), stop=(c == KP - 1),
                                     perf_mode=DR)
            if bqkv_sb is not None:
                nc.scalar.activation(out=qkT[:, m, :], in_=ps, func=AF.Identity,
                                     bias=bqkv_sb[:, m:m + 1], scale=1.0 / WS)
            else:
                nc.scalar.mul(qkT[:, m, :], ps, 1.0 / WS)

        ln1.release()

        # wfc1 prefetch overlaps the long attention stretch.
        p4 = tc.alloc_tile_pool(name="p4", bufs=1, side="left")
        x2lnT = p4.tile([P, KC, N], FP8)
        wfc1_sb = p4.tile([P, KC, HID], FP8)
        for ko in range(KC):
            nc.sync.dma_start(out=wfc1_sb[:, ko, :],
                              in_=wfc1_d.ap()[ko * P:(ko + 1) * P, :])

        # ---------------------------------------------------------------
        # Phase 3: attention. S bf16, exp -> fp8 probs (x64), AV fp8
        # DoubleRow over kt pairs, batched reciprocals per 6 heads.
        # ---------------------------------------------------------------
        p3 = tc.alloc_tile_pool(name="p3", bufs=1, side="left")
        att = tc.alloc_tile_pool(name="att", bufs=1, side="left")
        attnT = p3.tile([P, KC, N], FP8)
        wproj_sb = p3.tile([P, KC, C], FP8)
        for ko in range(KC):
            nc.sync.dma_start(out=wproj_sb[:, ko, :],
                              in_=wproj_d.ap()[ko * P:(ko + 1) * P, :])

        denom_d = dram.tile([H, N], F32, tag="denom", bufs=1)
        recip_d = dram.tile([H, N], F32, tag="recip", bufs=1)
        av_store = {}

        def attend_head(h):
            """S + exp + AV for head h; denominator row -> denom_d[h]."""
            pb = (h % 2) * HD
            qT = qkT[pb:pb + HD, h // 2, :]
            kT = qkT[pb:pb + HD, KC + h // 2, :]
            av = psum.tile([HD + 1, 1024], F32, tag="avp", bufs=2,
                           name=f"av_{h}")
            for ktp in range(NT // 2):
                es = att.tile([P, 2, N], FP8, tag="es", bufs=3,
                              name=f"es_{h}_{ktp}")
                for j in range(2):
                    kt = 2 * ktp + j
                    sps = psum.tile([P, 1024], F32, tag="mm", bufs=2,
                                    name=f"s_{h}_{kt}")
                    for q0 in (0, 512):
                        nc.tensor.matmul(sps[:, q0:q0 + 512],
                                         kT[:, kt * P:(kt + 1) * P],
                                         qT[:, q0:q0 + 512],
                                         start=True, stop=True)
                    # 64*exp(S/8) -> fp8, one [128, 1024] activation
                    nc.scalar.activation(out=es[:, j, :], in_=sps, func=AF.Exp,
                                         scale=0.125, bias=ln64_t)
                for q0 in (0, 512):
                    nc.tensor.matmul(av[:, q0:q0 + 512],
                                     v_aug[:, ktp, :, h * HB:h * HB + HD + 1],
                                     es[:, :, q0:q0 + 512],
                                     start=(ktp == 0), stop=(ktp == NT // 2 - 1),
                                     perf_mode=DR)
            av_sb = att.tile([HD + 1, N], F32, tag="avsb", bufs=H,
                             name=f"avsb_{h}")
            nc.vector.tensor_copy(out=av_sb, in_=av)
            nc.sync.dma_start(out=denom_d[h:h + 1, :], in_=av_sb[HD:HD + 1, :])
            av_store[h] = av_sb

        def normalize_heads(hs):
            """Batched reciprocal of denominators, then per-head normalize."""
            den = att.tile([len(hs), N], F32, tag="den", bufs=2)
            nc.sync.dma_start(out=den, in_=denom_d[hs[0]:hs[-1] + 1, :])
            nc.vector.reciprocal(out=den, in_=den)
            nc.sync.dma_start(out=recip_d[hs[0]:hs[-1] + 1, :], in_=den)
            for h in hs:
                rrow = recip_d[h, :]
                rbc_src = bass.AP(tensor=rrow.tensor, offset=rrow.offset,
                                  ap=[[0, HD]] + list(rrow.ap))
                rbc = att.tile([HD, N], F32, tag="rbc", bufs=2, name=f"rbc{h}")
                nc.sync.dma_start(out=rbc, in_=rbc_src)
                bounce = att.tile([HD, N], FP8, tag="bounce", bufs=2,
                                  name=f"bounce{h}")
                nc.vector.tensor_mul(out=bounce, in0=av_store[h][0:HD, :],
                                     in1=rbc)
                pb = (h % 2) * HD
                nc.sync.dma_start(out=attnT[pb:pb + HD, h // 2, :], in_=bounce)
                del av_store[h]

        for i in range(KC):
            project_qk(i)          # q slices for heads 2i, 2i+1
            project_qk(KC + i)     # k slices for heads 2i, 2i+1
            attend_head(2 * i)
            attend_head(2 * i + 1)
            if i == 2:
                normalize_heads(list(range(0, 6)))
        normalize_heads(list(range(6, H)))

        p2.release()

        # ---------------------------------------------------------------
        # Phase 4: proj (fp8 DoubleRow) + residual, LN2 -> x2_lnT
        # ---------------------------------------------------------------
        ln2 = tc.alloc_tile_pool(name="ln2", bufs=3, side="right")

        for t in range(NT):
            ps = psum.tile([P, 1024], F32, tag="mm", bufs=2, name="ps_pr")
            for n0, nn in ((0, 512), (512, 256)):
                for c in range(KP):
                    nc.tensor.matmul(ps[:, n0:n0 + nn],
                                     attnT[:, 2 * c:2 * c + 2, t * P:(t + 1) * P],
                                     wproj_sb[:, 2 * c:2 * c + 2, n0:n0 + nn],
                                     start=(c == 0), stop=(c == KP - 1),
                                     perf_mode=DR)
            ytmp = ln2.tile([P, C], BF16, tag="ytmp", bufs=2)
            nc.scalar.mul(ytmp, ps[:, 0:C], 1.0 / WS)
            xs = x_sb[:, t, :]
            nc.vector.tensor_add(out=xs, in0=xs, in1=ytmp)
            if "bproj" in g_beta:
                nc.vector.tensor_add(out=xs, in0=xs, in1=g_beta["bproj"])
            xln = layernorm_tile(ln2, xs, g_beta.get("g2"),
                                 g_beta.get("beta2"), "ln2")
            transpose_to(xln, x2lnT, t)

        att.release()
        p3.release()

        # ---------------------------------------------------------------
        # Phase 5: fc1 (fp8 DoubleRow) + gelu -> h^T (feature-major bf16)
        # ---------------------------------------------------------------
        p5 = tc.alloc_tile_pool(name="p5", bufs=1, side="left")
        hT = p5.tile([P, KH, N], BF16)
        wfc2_sb = p5.tile([P, KH, C], BF16)
        for ko in range(KH):
            nc.sync.dma_start(out=wfc2_sb[:, ko, :],
                              in_=wfc2_d.ap()[ko * P:(ko + 1) * P, :])

        for m in range(KH):
            ps = psum.tile([P, 1024], F32, tag="mm", bufs=2, name="ps_f1")
            for n0 in (0, 512):
                for c in range(KP):
                    nc.tensor.matmul(ps[:, n0:n0 + 512],
                                     wfc1_sb[:, 2 * c:2 * c + 2,
                                             m * P:(m + 1) * P],
                                     x2lnT[:, 2 * c:2 * c + 2, n0:n0 + 512],
                                     start=(c == 0), stop=(c == KP - 1),
                                     perf_mode=DR)
            bias = bfc1_sb[:, m:m + 1] if bfc1_sb is not None else 0.0
            nc.scalar.activation(out=hT[:, m, :], in_=ps, func=AF.Gelu,
                                 bias=bias, scale=1.0 / WS)

        # ---------------------------------------------------------------
        # Phase 6: fc2 (bf16) + residual -> out
        # ---------------------------------------------------------------
        for t in range(NT):
            ps = psum.tile([P, 1024], F32, tag="mm", bufs=2, name="ps_f2")
            for ko in range(KH):
                for n0, nn in ((0, 512), (512, 256)):
                    nc.tensor.matmul(ps[:, n0:n0 + nn],
                                     hT[:, ko, t * P:(t + 1) * P],
                                     wfc2_sb[:, ko, n0:n0 + nn],
                                     start=(ko == 0), stop=(ko == KH - 1))
            xs = x_sb[:, t, :]
            nc.vector.tensor_add(out=xs, in0=xs, in1=ps[:, 0:C])
            if "bfc2" in g_beta:
                nc.vector.tensor_add(out=xs, in0=xs, in1=g_beta["bfc2"])
            nc.sync.dma_start(out=out_d.ap()[t * P:(t + 1) * P, :],
                              in_=x_sb[:, t, :])

        p5.release()
        ln2.release()
        p4.release()
        p1.release()
        persist.release()
        dram.release()
        psum.release()

    nc.compile()
    return nc


def _prep(inputs):
    """Host-side prep: shard x over B, cast weights (fp8 x512 / bf16)."""
    f = {k: np.asarray(v) for k, v in inputs.items()}
    bf = ml_dtypes.bfloat16
    f8 = ml_dtypes.float8_e4m3

    def q8w(w):
        return np.ascontiguousarray(
            np.clip(w.astype(np.float32) * WS, -240, 240).astype(f8))

    flags = (
        bool(np.any(f["b_qkv"])),
        not np.all(f["g1"] == 1.0),
        bool(np.any(f["beta1"])),
        not np.all(f["g2"] == 1.0),
        bool(np.any(f["beta2"])),
        bool(np.any(f["b_fc1"])),
        bool(np.any(f["b_proj"])),
        bool(np.any(f["b_fc2"])),
    )
    (use_bqkv, use_g1, use_beta1, use_g2, use_beta2, use_bfc1, use_bproj,
     use_bfc2) = flags

    common = {
        "wqkv": q8w(f["w_qkv"]),
        "wproj": q8w(f["w_proj"]),
        "wfc1": q8w(f["w_fc1"]),
        "wfc2": np.ascontiguousarray(f["w_fc2"].astype(bf)),
    }
    for name, key, use in (
        ("bqkv", "b_qkv", use_bqkv), ("g1", "g1", use_g1),
        ("beta1", "beta1", use_beta1), ("g2", "g2", use_g2),
        ("beta2", "beta2", use_beta2), ("bfc1", "b_fc1", use_bfc1),
        ("bproj", "b_proj", use_bproj), ("bfc2", "b_fc2", use_bfc2),
    ):
        if use:
            common[name] = np.ascontiguousarray(f[key].astype(np.float32))

    x = f["x"].astype(np.float32)
    in_maps = [dict(common, x=np.ascontiguousarray(x[i])) for i in range(B)]
    return flags, in_maps


LAST_RESULT = None


def kernel(**inputs):
    global LAST_RESULT
    flags, in_maps = _prep(inputs)
    if flags not in _cache:
        _cache[flags] = _build(flags)
    nc = _cache[flags]
    res = bass_utils.run_bass_kernel_spmd(nc, in_maps, core_ids=list(range(B)))
    LAST_RESULT = res
    out = np.stack([r["out"] for r in res.results], axis=0)
    return out.astype(np.float32)


# revision 7
# speedup vs baseline: 1.3532x; 1.0720x over previous
"""Trainium2 Bass kernel for a dense transformer block.

Reference computation (per batch element):
    y  = Attention(LN1(x)) ; x = x + y
    x  = x + MLP(LN2(x))
with B=8, N=1024, C=768, H=12 heads, head_dim=64, HIDDEN=3072, fp32 I/O.

Sharding: data-parallel over B across the 8 NeuronCores — each core runs the
full block on one (1024, 768) batch element with replicated weights. No
collectives.

Per-core design notes:
  * Most matmuls run in fp8 (TRN FP8_EXP4 = e4m3, max +-240) with
    MatmulPerfMode.DoubleRow: operands carry TWO contraction chunks along a
    size-2 free dim ([K, 2, *]), so one instruction contracts 256 elements
    at bf16 instruction cost -> 2x PE throughput. Weights are pre-scaled by
    512 on the host (centers the 0.02-std weights in e4m3 range); the 1/512
    descale folds into each eviction (activation scale / tensor_scalar
    mult). fp8 matmuls: qkv, V, attention AV, proj, fc1.
  * S = q.k^T stays bf16 (its contraction dim is 64 - no pair layout without
    a partition shuffle) and fc2 stays bf16 for accuracy (fp8 fc1+fc2
    together would eat most of the 2e-2 error budget; fc1-only sims at
    1.33e-2 vs the 2e-2 gate).
  * Softmax: exp evictions are [128, 1024] activations spanning two PSUM
    banks; exp emits 64*exp(S/8) directly into fp8 (1/8 folded into the
    activation scale, the x64 into its bias = ln 64) so probabilities sit
    mid-range in e4m3. The AV matmul's 65th output row (from a ones-column
    in V) gives denominators; reciprocals are BATCHED across 6 heads into
    one [6, 1024] DVE reciprocal (DVE reciprocal cost depends only on free
    size - per-head [64, N] reciprocals waste 6.6us each).
  * qkT production is interleaved with per-head attention so the scalar
    engine's exp stream (the attention-phase bottleneck, ~100us) starts as
    soon as head 0's q/k slices exist instead of after all 12 projections.
  * Activations are token-major for layernorm/residuals; feature-major
    matmul operands come from PE transposes (bf16) whose evictions cast to
    fp8.
"""

import numpy as np
import ml_dtypes

import concourse.bass as bass
import concourse.bacc as bacc
import concourse.mybir as mybir
import concourse.tile as tile
from concourse import bass_utils

# Model dims (hardcoded per the problem spec).
B = 8
N = 1024  # tokens
C = 768  # model dim
H = 12  # heads
HD = 64  # head dim
HID = 3072  # mlp hidden
EPS = 1e-5
P = 128  # SBUF partitions

NT = N // P  # 8 token tiles
KC = C // P  # 6 contraction tiles over C
KP = KC // 2  # 3 DoubleRow pairs over C
KH = HID // P  # 24 contraction tiles over HIDDEN

WS = 512.0  # host-side weight scale for fp8 quantization
# Probability scale folded into the exp bias (probs = PS * exp(S/8) in fp8).
# Must keep PS * exp(max |S|/8) well under 240: TRN fp8 SATURATES TO INF, so
# an overflowing prob poisons the whole row. PS=8 trips only beyond ~11
# sigma of the score distribution while keeping typical probs ~[2, 30],
# mid-range in e4m3.
PS = 8.0

F32 = mybir.dt.float32
BF16 = mybir.dt.bfloat16
FP8 = mybir.dt.float8e4
AF = mybir.ActivationFunctionType
ALU = mybir.AluOpType
DR = mybir.MatmulPerfMode.DoubleRow

_cache = {}


def _build(flags):
    """Trace the per-core Bass program. `flags` gates optional bias/gain work."""
    (use_bqkv, use_g1, use_beta1, use_g2, use_beta2, use_bfc1, use_bproj,
     use_bfc2) = flags

    nc = bacc.Bacc("TRN2", target_bir_lowering=False, debug=False)

    x_d = nc.dram_tensor("x", [N, C], F32, kind="ExternalInput")
    wqkv_d = nc.dram_tensor("wqkv", [C, 3 * C], FP8, kind="ExternalInput")
    wproj_d = nc.dram_tensor("wproj", [C, C], FP8, kind="ExternalInput")
    wfc1_d = nc.dram_tensor("wfc1", [C, HID], FP8, kind="ExternalInput")
    wfc2_d = nc.dram_tensor("wfc2", [HID, C], BF16, kind="ExternalInput")
    out_d = nc.dram_tensor("out", [N, C], F32, kind="ExternalOutput")

    opt_d = {}
    for name, use, shape in (
        ("bqkv", use_bqkv, [3 * C]),
        ("g1", use_g1, [C]),
        ("beta1", use_beta1, [C]),
        ("g2", use_g2, [C]),
        ("beta2", use_beta2, [C]),
        ("bfc1", use_bfc1, [HID]),
        ("bproj", use_bproj, [C]),
        ("bfc2", use_bfc2, [C]),
    ):
        if use:
            opt_d[name] = nc.dram_tensor(name, shape, F32, kind="ExternalInput")

    def bcast_from_dram(pool, ap_1d, n):
        """[n] DRAM vector -> [P, n] SBUF tile replicated on every partition."""
        t = pool.tile([P, n], F32, name=f"bc_{ap_1d.tensor.name}")
        src = bass.AP(tensor=ap_1d.tensor, offset=ap_1d.offset,
                      ap=[[0, P]] + list(ap_1d.ap))
        nc.sync.dma_start(out=t, in_=src)
        return t

    with tile.TileContext(nc) as tc:
        persist = tc.alloc_tile_pool(name="persist", bufs=1, side="left")
        psum = tc.alloc_tile_pool(name="psum", bufs=1, space="PSUM")
        dram = tc.alloc_tile_pool(name="dram", bufs=2, space="DRAM")

        # Residual stream, token-major; updated in place through the block.
        x_sb = persist.tile([P, NT, C], F32)
        for t in range(NT):
            nc.sync.dma_start(out=x_sb[:, t, :], in_=x_d.ap()[t * P:(t + 1) * P, :])
        eps_t = persist.tile([P, 1], F32)
        nc.vector.memset(eps_t, EPS)
        ln64_t = persist.tile([P, 1], F32)
        nc.vector.memset(ln64_t, float(np.log(PS)))

        # Identity (bf16, embedded in the NEFF) for PE-based transposes.
        ident_d = nc.inline_tensor(np.eye(P, dtype=ml_dtypes.bfloat16), "ident")
        ident = persist.tile([P, P], BF16)
        nc.sync.dma_start(out=ident, in_=ident_d.ap())

        g_beta = {}
        for name, n in (("g1", C), ("beta1", C), ("g2", C), ("beta2", C),
                        ("bproj", C), ("bfc2", C)):
            if name in opt_d:
                g_beta[name] = bcast_from_dram(persist, opt_d[name].ap(), n)
        bqkv_sb = None
        if "bqkv" in opt_d:
            bqkv_sb = persist.tile([P, 3 * C // P], F32)
            nc.sync.dma_start(out=bqkv_sb,
                              in_=opt_d["bqkv"].ap().rearrange("(m p) -> p m", p=P))
        bfc1_sb = None
        if "bfc1" in opt_d:
            bfc1_sb = persist.tile([P, KH], F32)
            nc.sync.dma_start(out=bfc1_sb,
                              in_=opt_d["bfc1"].ap().rearrange("(m p) -> p m", p=P))

        # ---------------------------------------------------------------
        # Phase 1: LN1 (token-major) -> x_lnT (feature-major fp8), weights
        # ---------------------------------------------------------------
        p1 = tc.alloc_tile_pool(name="p1", bufs=1, side="left")
        ln1 = tc.alloc_tile_pool(name="ln1", bufs=3, side="left")

        wqkv_sb = p1.tile([P, KC, 3 * C], FP8)
        for ko in range(KC):
            nc.sync.dma_start(out=wqkv_sb[:, ko, :],
                              in_=wqkv_d.ap()[ko * P:(ko + 1) * P, :])

        xlnT = p1.tile([P, KC, N], FP8)

        def layernorm_tile(pool, x_ap, g_sb, beta_sb, name):
            """x_ap: [P, C] fp32 token-major -> returns [P, C] bf16 tile."""
            stats = pool.tile([P, 3, 6], F32, tag=f"{name}_st", bufs=3)
            xr = x_ap.rearrange("p (s f) -> p s f", f=256)
            for s in range(3):
                nc.vector.bn_stats(out=stats[:, s, :], in_=xr[:, s, :])
            mv = pool.tile([P, 2], F32, tag=f"{name}_mv", bufs=3)
            nc.vector.bn_aggr(out=mv, in_=stats)
            rstd = pool.tile([P, 1], F32, tag=f"{name}_rs", bufs=3)
            nc.scalar.activation(out=rstd, in_=mv[:, 1:2], func=AF.Sqrt,
                                 bias=eps_t, scale=1.0)
            nc.vector.reciprocal(out=rstd, in_=rstd)
            xln = pool.tile([P, C], BF16, tag=f"{name}_xln", bufs=3)
            nc.vector.tensor_scalar(out=xln, in0=x_ap, scalar1=mv[:, 0:1],
                                    scalar2=rstd, op0=ALU.subtract, op1=ALU.mult)
            if g_sb is not None:
                nc.vector.tensor_mul(out=xln, in0=xln, in1=g_sb)
            if beta_sb is not None:
                nc.vector.tensor_add(out=xln, in0=xln, in1=beta_sb)
            return xln

        def transpose_to(xln, dstT, t):
            """[P, C] token-major bf16 tile -> dstT[:, :, t*P:(t+1)*P] fp8."""
            for c in range(KC):
                tps = psum.tile([P, P], BF16, tag="avp", bufs=2, name="tps")
                nc.tensor.transpose(tps, xln[:, c * P:(c + 1) * P], ident)
                nc.scalar.copy(out=dstT[:, c, t * P:(t + 1) * P], in_=tps)

        for t in range(NT):
            xln = layernorm_tile(ln1, x_sb[:, t, :], g_beta.get("g1"),
                                 g_beta.get("beta1"), "ln1")
            transpose_to(xln, xlnT, t)

        # ---------------------------------------------------------------
        # Phase 2: V projection first (fp8 DoubleRow), then qkT interleaved
        # with attention heads.
        #   V token-major fp8, kt-paired with a ones column:
        #     v_aug [P, NT/2, 2, H*(HD+1)]
        # ---------------------------------------------------------------
        p2 = tc.alloc_tile_pool(name="p2", bufs=1, side="right")
        qkT = p2.tile([P, 2 * KC, N], BF16)
        # Head blocks padded to 80 bytes: dual-fp8 LDWEIGHTS requires the
        # pair-dim byte stride (2*H*HB) and offsets to be 16-aligned.
        HB = 80  # 64 v-cols + 1 ones column + 15 pad
        v_aug = p2.tile([P, NT // 2, 2, H * HB], FP8)
        ones_view = v_aug.rearrange("p a b (h d) -> p (a b h) d", d=HB)
        nc.vector.memset(ones_view[:, :, HD:HD + 1], 1.0)

        bq_v = None
        if use_bqkv:
            bq_v = bcast_from_dram(persist, opt_d["bqkv"].ap()[2 * C:3 * C], C)

        for t in range(NT):
            ps = psum.tile([P, 1024], F32, tag="mm", bufs=2, name="ps_v")
            for n0, nn in ((0, 512), (512, 256)):
                for c in range(KP):
                    nc.tensor.matmul(ps[:, n0:n0 + nn],
                                     xlnT[:, 2 * c:2 * c + 2, t * P:(t + 1) * P],
                                     wqkv_sb[:, 2 * c:2 * c + 2,
                                             2 * C + n0:2 * C + n0 + nn],
                                     start=(c == 0), stop=(c == KP - 1),
                                     perf_mode=DR)
            # scatter heads into the pair-major V_aug layout (fp8, 1/WS)
            dst = v_aug[:, t // 2, t % 2, :].rearrange(
                "p (h d) -> p h d", d=HB)[:, :, 0:HD]
            if bq_v is not None:
                tmp = ln1.tile([P, C], F32, tag="vtmp", bufs=2)
                nc.vector.tensor_scalar_mul(tmp, ps[:, 0:C], 1.0 / WS)
                nc.vector.tensor_add(out=tmp, in0=tmp, in1=bq_v)
                nc.vector.tensor_copy(
                    out=dst, in_=tmp.rearrange("p (h d) -> p h d", d=HD))
            else:
                nc.vector.tensor_scalar_mul(
                    dst, ps[:, 0:C].rearrange("p (h d) -> p h d", d=HD),
                    1.0 / WS)

        def project_qk(m):
            """qkT[:, m, :] = (wqkv[:, m-block].T @ x_ln^T) / WS (+bias)."""
            ps = psum.tile([P, 1024], F32, tag="mm", bufs=2, name="ps_qk")
            for n0 in (0, 512):
                for c in range(KP):
                    nc.tensor.matmul(ps[:, n0:n0 + 512],
                                     wqkv_sb[:, 2 * c:2 * c + 2,
                                             m * P:(m + 1) * P],
                                     xlnT[:, 2 * c:2 * c + 2, n0:n0 + 512],
                                     start=(c == 0), stop=(c == KP - 1),
                                     perf_mode=DR)
            if bqkv_sb is not None:
                nc.scalar.activation(out=qkT[:, m, :], in_=ps, func=AF.Identity,
                                     bias=bqkv_sb[:, m:m + 1], scale=1.0 / WS)
            else:
                nc.scalar.mul(qkT[:, m, :], ps, 1.0 / WS)

        ln1.release()

        # wfc1 prefetch overlaps the long attention stretch.
        p4 = tc.alloc_tile_pool(name="p4", bufs=1, side="left")
        x2lnT = p4.tile([P, KC, N], FP8)
        wfc1_sb = p4.tile([P, KC, HID], FP8)
        for ko in range(KC):
            nc.sync.dma_start(out=wfc1_sb[:, ko, :],
                              in_=wfc1_d.ap()[ko * P:(ko + 1) * P, :])

        # ---------------------------------------------------------------
        # Phase 3: attention. S bf16, exp -> fp8 probs (x64), AV fp8
        # DoubleRow over kt pairs, batched reciprocals per 6 heads.
        # ---------------------------------------------------------------
        p3 = tc.alloc_tile_pool(name="p3", bufs=1, side="left")
        att = tc.alloc_tile_pool(name="att", bufs=1, side="left")
        attnT = p3.tile([P, KC, N], FP8)
        wproj_sb = p3.tile([P, KC, C], FP8)
        for ko in range(KC):
            nc.sync.dma_start(out=wproj_sb[:, ko, :],
                              in_=wproj_d.ap()[ko * P:(ko + 1) * P, :])

        denom_d = dram.tile([H, N], F32, tag="denom", bufs=1)
        recip_d = dram.tile([H, N], F32, tag="recip", bufs=1)
        av_store = {}

        NKTP = NT // 2
        pending_av = []  # [(h, ktp, av_psum, es)] not yet issued to the PE

        def issue_av(count=1):
            """Issue up to `count` deferred AV matmuls (oldest first)."""
            for _ in range(min(count, len(pending_av))):
                h, ktp, av, es = pending_av.pop(0)
                for q0 in (0, 512):
                    nc.tensor.matmul(av[:, q0:q0 + 512],
                                     v_aug[:, ktp, :, h * HB:h * HB + HD + 1],
                                     es[:, :, q0:q0 + 512],
                                     start=(ktp == 0), stop=(ktp == NKTP - 1),
                                     perf_mode=DR)
                if ktp == NKTP - 1:
                    av_sb = att.tile([HD + 1, N], F32, tag="avsb", bufs=H,
                                     name=f"avsb_{h}")
                    nc.vector.tensor_copy(out=av_sb, in_=av)
                    nc.sync.dma_start(out=denom_d[h:h + 1, :],
                                      in_=av_sb[HD:HD + 1, :])
                    av_store[h] = av_sb

        def attend_head(h):
            """S + exp for head h; AV matmuls trail one kt-pair behind so the
            PE never waits on the exp that feeds them."""
            pb = (h % 2) * HD
            qT = qkT[pb:pb + HD, h // 2, :]
            kT = qkT[pb:pb + HD, KC + h // 2, :]
            av = psum.tile([HD + 1, 1024], F32, tag="avp", bufs=2,
                           name=f"av_{h}")
            for ktp in range(NKTP):
                es = att.tile([P, 2, N], FP8, tag="es", bufs=4,
                              name=f"es_{h}_{ktp}")
                for j in range(2):
                    kt = 2 * ktp + j
                    sps = psum.tile([P, 1024], F32, tag="mm", bufs=2,
                                    name=f"s_{h}_{kt}")
                    for q0 in (0, 512):
                        nc.tensor.matmul(sps[:, q0:q0 + 512],
                                         kT[:, kt * P:(kt + 1) * P],
                                         qT[:, q0:q0 + 512],
                                         start=True, stop=True)
                    # PS*exp(S/8) -> fp8, one [128, 1024] activation
                    nc.scalar.activation(out=es[:, j, :], in_=sps, func=AF.Exp,
                                         scale=0.125, bias=ln64_t)
                pending_av.append((h, ktp, av, es))
                if len(pending_av) > 1:
                    issue_av(1)

        def normalize_heads(hs):
            """Batched reciprocal of denominators, then per-head normalize.
            The group's [m, N] denominators are reshaped partition-major to
            [128, m*8] via DRAM so one DVE reciprocal costs ~m*8 elem/lane
            instead of N (reciprocal time scales with free size only)."""
            m = len(hs)
            fold = (m * N) // P
            den = att.tile([P, fold], F32, tag="den", bufs=2)
            src_f = denom_d[hs[0]:hs[-1] + 1, :].rearrange(
                "h q -> (h q)").rearrange("(p c) -> p c", p=P)
            nc.sync.dma_start(out=den, in_=src_f)
            nc.vector.reciprocal(out=den, in_=den)
            dst_f = recip_d[hs[0]:hs[-1] + 1, :].rearrange(
                "h q -> (h q)").rearrange("(p c) -> p c", p=P)
            nc.sync.dma_start(out=dst_f, in_=den)
            for h in hs:
                rrow = recip_d[h, :]
                rbc_src = bass.AP(tensor=rrow.tensor, offset=rrow.offset,
                                  ap=[[0, HD]] + list(rrow.ap))
                rbc = att.tile([HD, N], F32, tag="rbc", bufs=2, name=f"rbc{h}")
                nc.sync.dma_start(out=rbc, in_=rbc_src)
                bounce = att.tile([HD, N], FP8, tag="bounce", bufs=2,
                                  name=f"bounce{h}")
                nc.vector.tensor_mul(out=bounce, in0=av_store[h][0:HD, :],
                                     in1=rbc)
                pb = (h % 2) * HD
                nc.sync.dma_start(out=attnT[pb:pb + HD, h // 2, :], in_=bounce)
                del av_store[h]

        for i in range(KC):
            project_qk(i)          # q slices for heads 2i, 2i+1
            project_qk(KC + i)     # k slices for heads 2i, 2i+1
            attend_head(2 * i)
            attend_head(2 * i + 1)
            if i == 3:
                issue_av(1)        # flush head 7 fully
                normalize_heads(list(range(0, 8)))
        issue_av(len(pending_av))
        normalize_heads(list(range(8, H)))

        p2.release()

        # ---------------------------------------------------------------
        # Phase 4: proj (fp8 DoubleRow) + residual, LN2 -> x2_lnT
        # ---------------------------------------------------------------
        ln2 = tc.alloc_tile_pool(name="ln2", bufs=3, side="right")

        for t in range(NT):
            ps = psum.tile([P, 1024], F32, tag="mm", bufs=2, name="ps_pr")
            for n0, nn in ((0, 512), (512, 256)):
                for c in range(KP):
                    nc.tensor.matmul(ps[:, n0:n0 + nn],
                                     attnT[:, 2 * c:2 * c + 2, t * P:(t + 1) * P],
                                     wproj_sb[:, 2 * c:2 * c + 2, n0:n0 + nn],
                                     start=(c == 0), stop=(c == KP - 1),
                                     perf_mode=DR)
            ytmp = ln2.tile([P, C], BF16, tag="ytmp", bufs=2)
            nc.scalar.mul(ytmp, ps[:, 0:C], 1.0 / WS)
            xs = x_sb[:, t, :]
            nc.vector.tensor_add(out=xs, in0=xs, in1=ytmp)
            if "bproj" in g_beta:
                nc.vector.tensor_add(out=xs, in0=xs, in1=g_beta["bproj"])
            xln = layernorm_tile(ln2, xs, g_beta.get("g2"),
                                 g_beta.get("beta2"), "ln2")
            transpose_to(xln, x2lnT, t)

        att.release()
        p3.release()

        # ---------------------------------------------------------------
        # Phase 5: fc1 (fp8 DoubleRow) + gelu -> h^T (feature-major bf16)
        # ---------------------------------------------------------------
        p5 = tc.alloc_tile_pool(name="p5", bufs=1, side="left")
        hT = p5.tile([P, KH, N], BF16)
        wfc2_sb = p5.tile([P, KH, C], BF16)
        for ko in range(KH):
            nc.sync.dma_start(out=wfc2_sb[:, ko, :],
                              in_=wfc2_d.ap()[ko * P:(ko + 1) * P, :])

        for m in range(KH):
            ps = psum.tile([P, 1024], F32, tag="mm", bufs=2, name="ps_f1")
            for n0 in (0, 512):
                for c in range(KP):
                    nc.tensor.matmul(ps[:, n0:n0 + 512],
                                     wfc1_sb[:, 2 * c:2 * c + 2,
                                             m * P:(m + 1) * P],
                                     x2lnT[:, 2 * c:2 * c + 2, n0:n0 + 512],
                                     start=(c == 0), stop=(c == KP - 1),
                                     perf_mode=DR)
            bias = bfc1_sb[:, m:m + 1] if bfc1_sb is not None else 0.0
            nc.scalar.activation(out=hT[:, m, :], in_=ps, func=AF.Gelu,
                                 bias=bias, scale=1.0 / WS)

        # ---------------------------------------------------------------
        # Phase 6: fc2 (bf16) + residual -> out
        # ---------------------------------------------------------------
        for t in range(NT):
            ps = psum.tile([P, 1024], F32, tag="mm", bufs=2, name="ps_f2")
            for ko in range(KH):
                for n0, nn in ((0, 512), (512, 256)):
                    nc.tensor.matmul(ps[:, n0:n0 + nn],
                                     hT[:, ko, t * P:(t + 1) * P],
                                     wfc2_sb[:, ko, n0:n0 + nn],
                                     start=(ko == 0), stop=(ko == KH - 1))
            xs = x_sb[:, t, :]
            nc.vector.tensor_add(out=xs, in0=xs, in1=ps[:, 0:C])
            if "bfc2" in g_beta:
                nc.vector.tensor_add(out=xs, in0=xs, in1=g_beta["bfc2"])
            nc.sync.dma_start(out=out_d.ap()[t * P:(t + 1) * P, :],
                              in_=x_sb[:, t, :])

        p5.release()
        ln2.release()
        p4.release()
        p1.release()
        persist.release()
        dram.release()
        psum.release()

    nc.compile()
    return nc


def _prep(inputs):
    """Host-side prep: shard x over B, cast weights (fp8 x512 / bf16)."""
    f = {k: np.asarray(v) for k, v in inputs.items()}
    bf = ml_dtypes.bfloat16
    f8 = ml_dtypes.float8_e4m3

    def q8w(w):
        return np.ascontiguousarray(
            np.clip(w.astype(np.float32) * WS, -240, 240).astype(f8))

    flags = (
        bool(np.any(f["b_qkv"])),
        not np.all(f["g1"] == 1.0),
        bool(np.any(f["beta1"])),
        not np.all(f["g2"] == 1.0),
        bool(np.any(f["beta2"])),
        bool(np.any(f["b_fc1"])),
        bool(np.any(f["b_proj"])),
        bool(np.any(f["b_fc2"])),
    )
    (use_bqkv, use_g1, use_beta1, use_g2, use_beta2, use_bfc1, use_bproj,
     use_bfc2) = flags

    common = {
        "wqkv": q8w(f["w_qkv"]),
        "wproj": q8w(f["w_proj"]),
        "wfc1": q8w(f["w_fc1"]),
        "wfc2": np.ascontiguousarray(f["w_fc2"].astype(bf)),
    }
    for name, key, use in (
        ("bqkv", "b_qkv", use_bqkv), ("g1", "g1", use_g1),
        ("beta1", "beta1", use_beta1), ("g2", "g2", use_g2),
        ("beta2", "beta2", use_beta2), ("bfc1", "b_fc1", use_bfc1),
        ("bproj", "b_proj", use_bproj), ("bfc2", "b_fc2", use_bfc2),
    ):
        if use:
            common[name] = np.ascontiguousarray(f[key].astype(np.float32))

    x = f["x"].astype(np.float32)
    in_maps = [dict(common, x=np.ascontiguousarray(x[i])) for i in range(B)]
    return flags, in_maps


LAST_RESULT = None


def kernel(**inputs):
    global LAST_RESULT
    flags, in_maps = _prep(inputs)
    if flags not in _cache:
        _cache[flags] = _build(flags)
    nc = _cache[flags]
    res = bass_utils.run_bass_kernel_spmd(nc, in_maps, core_ids=list(range(B)))
    LAST_RESULT = res
    out = np.stack([r["out"] for r in res.results], axis=0)
    return out.astype(np.float32)
